# revision 8
# baseline (speedup 1.0000x reference)
"""Causal self-attention (B=4, T=2048, C=1024, H=16) on 8 TRN2 NeuronCores.

Sharding: core = (batch, head-group) — data parallel over the 4 batches,
tensor parallel over 2 groups of 8 heads (Megatron-style column/row split of
the qkv / out projections).  Each core computes a [T, C] partial of the out
projection for its head group; the host sums the two partials per batch and
adds b_out, so no device collectives are needed.

v2 (vs v1): all matmul operands in bf16 (fp32 PSUM accumulation), the two
heads of a pair share one [128, 2x512] PSUM S-tile so each block needs a
single wide Exp on ScalarE, the causal mask is applied pre-exp by an
accumulating identity @ (-1e30 * strict-lower) matmul (no GpSimd hop), and
psY is evicted to SBUF immediately so the next pair's PV accumulation never
waits on the normalize / DRAM-bounce broadcast tail.
"""

import os
import sys
from contextlib import ExitStack

import numpy as np

for _p in ("/opt/trn_rl_repo", "/root/.axon_site/_ro/trn_rl_repo"):
    if os.path.isdir(_p) and _p not in sys.path:
        sys.path.append(_p)

import concourse.bacc as bacc
import concourse.bass as bass
import concourse.tile as tile
from concourse import mybir
from concourse.bass_utils import run_bass_kernel_spmd
from concourse.masks import make_identity, make_upper_triangular

AF = mybir.ActivationFunctionType
ALU = mybir.AluOpType
F32 = mybir.dt.float32
F32R = mybir.dt.float32r
BF16 = mybir.dt.bfloat16

P = 128
SLAB = 512

B, T, C, H, D = 4, 2048, 1024, 16, 64
N_CORES = 8
N_GROUPS = 2          # head groups (tensor-parallel degree per batch)
HL = H // N_GROUPS    # heads per core
CL = HL * D           # local qkv width


def _build_nc(loop_reps=None):
    NCK = C // P
    MQK = 2 * CL // P
    MQ = MQK // 2
    TT = T // P
    NS = T // SLAB
    YC = CL // P
    W_OUT = min(SLAB, C)
    NOUT = C // W_OUT
    scale = 1.0 / np.sqrt(D)

    nc = bacc.Bacc("TRN2", target_bir_lowering=False, debug=False,
                   num_devices=N_CORES)
    xT = nc.dram_tensor("xT", [C, T], BF16, kind="ExternalInput")
    wqk = nc.dram_tensor("wqk", [C, 2 * CL], BF16, kind="ExternalInput")
    wv = nc.dram_tensor("wv", [C, CL], BF16, kind="ExternalInput")
    wout = nc.dram_tensor("wout", [CL, C], BF16, kind="ExternalInput")
    bqk = nc.dram_tensor("bqk", [P, MQK], F32, kind="ExternalInput")
    bv = nc.dram_tensor("bv", [1, CL], BF16, kind="ExternalInput")
    outp = nc.dram_tensor("outp", [T, C], BF16, kind="ExternalOutput")

    with tile.TileContext(nc) as tc, ExitStack() as ctx:
        pool = lambda name, bufs, **kw: ctx.enter_context(
            tc.tile_pool(name=name, bufs=bufs, **kw))

        const = pool("const", 1)
        kp = pool("kp", 1)
        vp = pool("vp", 1)
        wqkp = pool("wqkp", 1)
        wvp = pool("wvp", 1)
        woutp = pool("woutp", 1)
        xtp = pool("xt", 2)
        qp = pool("qp", 2)
        yTp = pool("yTp", 2)
        expp = pool("expp", 3)
        yfp = pool("yfp", 2)
        nrm_b = pool("nrm_b", 2)
        y8p = pool("y8", 2)
        otp = pool("ot", 2)
        psS = pool("psS", 2, space="PSUM")       # [128, 2, 512] = 2 banks each
        psY = pool("psY", 1, space="PSUM")       # py0 + py1 = 2 banks
        psProj = pool("psProj", 1, space="PSUM")
        psO = pool("psO", 1, space="PSUM")

        k_sb = kp.tile([P, MQ, T], BF16)
        v_sb = vp.tile([P, TT, HL, D + 1], BF16)
        wqk_sb = wqkp.tile([P, NCK, 2 * CL], BF16)
        wv_sb = wvp.tile([P, NCK, CL], BF16)
        wout_sb = woutp.tile([P, YC, C], BF16)
        bqk_sb = const.tile([P, MQK], F32)
        bv_sb = const.tile([1, CL], BF16)
        ident = const.tile([P, P], BF16)
        maskneg = const.tile([P, P], BF16)
        mask01 = const.tile([P, P], F32)
        onescr = const.tile([P, TT * HL], F32)

        nc.sync.dma_start(out=bqk_sb[:, :], in_=bqk[:, :])
        nc.sync.dma_start(out=bv_sb[:, :], in_=bv[:, :])
        for c in range(NCK):
            nc.sync.dma_start(out=wqk_sb[:, c, :], in_=wqk[c * P:(c + 1) * P, :])
            nc.sync.dma_start(out=wv_sb[:, c, :], in_=wv[c * P:(c + 1) * P, :])
        for c in range(YC):
            nc.sync.dma_start(out=wout_sb[:, c, :], in_=wout[c * P:(c + 1) * P, :])
        # mask01[p, f] = 1 if f >= p else 0; maskneg = (mask01 - 1) * 1e30
        # i.e. -1e30 on the strictly-lower (tq < tk) region, 0 elsewhere.
        make_upper_triangular(nc, mask01[:, :], val=1.0, diag=True)
        nc.vector.tensor_scalar(
            maskneg[:, :], mask01[:, :], -1.0, 1e30, op0=ALU.add, op1=ALU.mult)
        make_identity(nc, mask01[:, :])
        nc.vector.tensor_copy(ident[:, :], mask01[:, :])
        # bf16 tiles can't be memset; memset f32 scratch + DVE copy (rounds)
        nc.vector.memset(onescr[:, :], 1.0)
        nc.vector.tensor_copy(
            v_sb[:, :, :, D],
            onescr[:, :].rearrange("p (t h) -> p t h", h=HL))
        ones1 = v_sb[0:1, :, :, D].rearrange("u t h -> u (t h)")

        def body():
            for s in range(NS):
                t0 = s * SLAB
                # ---- A(s): projections for this slab ----
                xt = xtp.tile([P, NCK, SLAB], BF16)
                for c in range(NCK):
                    nc.sync.dma_start(out=xt[:, c, :],
                                      in_=xT[c * P:(c + 1) * P, t0:t0 + SLAB])
                q_sb = qp.tile([P, MQ, SLAB], BF16)
                for m in range(MQK):
                    ps = psProj.tile([P, SLAB], F32, tag="ps")
                    for c in range(NCK):
                        nc.tensor.matmul(
                            ps[:, :],
                            wqk_sb[:, c, m * P:(m + 1) * P],
                            xt[:, c, :],
                            start=(c == 0), stop=(c == NCK - 1))
                    dst = (q_sb[:, m, :] if m < MQ
                           else k_sb[:, m - MQ, t0:t0 + SLAB])
                    sc = scale if m < MQ else 1.0
                    nc.vector.tensor_scalar(
                        dst, ps[:, :], sc, bqk_sb[:, m:m + 1],
                        op0=ALU.mult, op1=ALU.add)
                for sub in range(SLAB // P):
                    tt = s * (SLAB // P) + sub
                    ps = psProj.tile([P, CL], F32, tag="ps")
                    for c in range(NCK):
                        nc.tensor.matmul(
                            ps[:, :],
                            xt[:, c, sub * P:(sub + 1) * P],
                            wv_sb[:, c, :],
                            start=(c == 0), stop=False)
                    nc.tensor.matmul(
                        ps[:, :], ones1[:, :],
                        bv_sb[0:1, :], start=False, stop=True)
                    nc.vector.tensor_copy(
                        v_sb[:, tt, :, 0:D],
                        ps[:, :].rearrange("p (h d) -> p h d", d=D))

                # ---- B: attention; even/odd heads of a pair share the PE
                # array via tile_position row groups (concurrent K=64) and
                # share one 2-bank PSUM S-tile so one Exp covers both ----
                yT_sb = yTp.tile([P, YC, SLAB], BF16)
                for hp in range(HL // 2):
                    nblk = (s + 1) * SLAB // P
                    py0 = psY.tile([D + 1, SLAB], F32, tag="py0")
                    py1 = psY.tile([D + 1, SLAB], F32, tag="py1")
                    pys = (py0, py1)
                    for b in range(nblk):
                        tk0 = b * P
                        off = tk0 - t0
                        vis = max(0, off)
                        ss = psS.tile([P, 2, SLAB], F32)
                        for i in range(2):
                            row0 = i * 64
                            nc.tensor.matmul(
                                ss[:, i, vis:SLAB],
                                k_sb[row0:row0 + 64, hp, tk0:tk0 + P],
                                q_sb[row0:row0 + 64, hp, vis:SLAB],
                                start=True, stop=(off < 0),
                                tile_position=(row0, 0))
                        if off >= 0:
                            for i in range(2):
                                nc.tensor.matmul(
                                    ss[:, i, off:off + P],
                                    ident[:, :], maskneg[:, :],
                                    start=False, stop=True)
                        ep = expp.tile([P, 2, SLAB], BF16)
                        nc.scalar.activation(ep[:, :, vis:SLAB],
                                             ss[:, :, vis:SLAB], AF.Exp)
                        for i in range(2):
                            nc.tensor.matmul(
                                pys[i][0:D + 1, vis:SLAB],
                                v_sb[:, b, 2 * hp + i, 0:D + 1],
                                ep[:, i, vis:SLAB],
                                start=(b == 0), stop=(b == nblk - 1))
                    # evict psY to SBUF right away so the next pair's PV
                    # accumulation doesn't wait on the normalize tail;
                    # reciprocal reads PSUM directly (parallel with the
                    # copy), broadcast + multiply run on the idle Pool
                    # engine (partition_broadcast is SBUF-only).
                    for i in range(2):
                        row0 = i * 64
                        yf = yfp.tile([D, SLAB], F32, tag="yf")
                        nc.vector.tensor_copy(yf[:, :], pys[i][0:D, :])
                        bi = nrm_b.tile([64, SLAB], F32, tag="binv")
                        nc.vector.reciprocal(bi[0:1, :], pys[i][D:D + 1, :])
                        nc.gpsimd.partition_broadcast(bi[:, :], bi[0:1, :])
                        y8 = y8p.tile([64, SLAB], BF16)
                        nc.gpsimd.tensor_mul(y8[:, :], yf[:, :], bi[:, :])
                        nc.sync.dma_start(
                            out=yT_sb[row0:row0 + 64, hp, :], in_=y8[:, :])

                # ---- C(s): out projection for this slab ----
                for sub in range(SLAB // P):
                    for n in range(NOUT):
                        n0 = n * W_OUT
                        ps = psO.tile([P, W_OUT], F32, tag="ps")
                        for c in range(YC):
                            nc.tensor.matmul(
                                ps[:, :],
                                yT_sb[:, c, sub * P:(sub + 1) * P],
                                wout_sb[:, c, n0:n0 + W_OUT],
                                start=(c == 0), stop=(c == YC - 1))
                        ot = otp.tile([P, W_OUT], BF16)
                        nc.vector.tensor_copy(ot[:, :], ps[:, :])
                        nc.sync.dma_start(
                            out=outp[t0 + sub * P:t0 + (sub + 1) * P,
                                     n0:n0 + W_OUT],
                            in_=ot[:, :])

        if loop_reps is None:
            body()
        else:
            with tc.For_i(0, loop_reps, 1):
                body()

    nc.compile()
    return nc


_NC_CACHE = None


def _get_nc():
    global _NC_CACHE
    if _NC_CACHE is None:
        _NC_CACHE = _build_nc()
    return _NC_CACHE


def make_in_maps(x, W_qkv, b_qkv, W_out):
    import ml_dtypes

    bf = ml_dtypes.bfloat16
    scale = 1.0 / np.sqrt(D)
    MQK = 2 * CL // P
    in_maps = []
    for core in range(N_CORES):
        b, hg = divmod(core, N_GROUPS)
        qs = slice(hg * CL, (hg + 1) * CL)
        ks = slice(C + hg * CL, C + (hg + 1) * CL)
        vs = slice(2 * C + hg * CL, 2 * C + (hg + 1) * CL)
        bqk_cat = np.concatenate([b_qkv[qs] * scale, b_qkv[ks]])
        in_maps.append({
            "xT": np.ascontiguousarray(x[b].T).astype(bf),
            "wqk": np.ascontiguousarray(
                np.concatenate([W_qkv[:, qs], W_qkv[:, ks]], axis=1)).astype(bf),
            "wv": np.ascontiguousarray(W_qkv[:, vs]).astype(bf),
            "wout": np.ascontiguousarray(W_out[hg * CL:(hg + 1) * CL, :]).astype(bf),
            "bqk": np.ascontiguousarray(bqk_cat.reshape(MQK, P).T),
            "bv": np.ascontiguousarray(b_qkv[vs].reshape(1, CL)).astype(bf),
        })
    return in_maps


def kernel(x, W_qkv, b_qkv, W_out, b_out):
    x = np.asarray(x, dtype=np.float32)
    W_qkv = np.asarray(W_qkv, dtype=np.float32)
    b_qkv = np.asarray(b_qkv, dtype=np.float32)
    W_out = np.asarray(W_out, dtype=np.float32)
    b_out = np.asarray(b_out, dtype=np.float32)

    nc = _get_nc()
    in_maps = make_in_maps(x, W_qkv, b_qkv, W_out)
    res = run_bass_kernel_spmd(nc, in_maps, core_ids=list(range(N_CORES)))

    out = np.empty((B, T, C), dtype=np.float32)
    for b in range(B):
        out[b] = (res.results[N_GROUPS * b]["outp"].astype(np.float32)
                  + res.results[N_GROUPS * b + 1]["outp"].astype(np.float32)
                  + b_out)
    return out


# revision 10
# speedup vs baseline: 1.7934x; 1.7934x over previous
"""Causal self-attention (B=4, T=2048, C=1024, H=16) on 8 TRN2 NeuronCores.

Sharding: core = (batch, head-group) — data parallel over the 4 batches,
tensor parallel over 2 groups of 8 heads (Megatron-style column/row split of
the qkv / out projections).  Each core computes a [T, C] partial of the out
projection for its head group; the host sums the two partials per batch and
adds b_out, so no device collectives are needed.

v2 (vs v1): all matmul operands in bf16 (fp32 PSUM accumulation), the two
heads of a pair share one [128, 2x512] PSUM S-tile so each block needs a
single wide Exp on ScalarE, the causal mask is applied pre-exp by an
accumulating identity @ (-1e30 * strict-lower) matmul (no GpSimd hop), and
psY is evicted to SBUF immediately so the next pair's PV accumulation never
waits on the normalize / DRAM-bounce broadcast tail.
"""

import os
import sys
from contextlib import ExitStack

import numpy as np

for _p in ("/opt/trn_rl_repo", "/root/.axon_site/_ro/trn_rl_repo"):
    if os.path.isdir(_p) and _p not in sys.path:
        sys.path.append(_p)

import concourse.bacc as bacc
import concourse.bass as bass
import concourse.tile as tile
from concourse import mybir
from concourse.bass_utils import run_bass_kernel_spmd
from concourse.masks import make_identity, make_upper_triangular

AF = mybir.ActivationFunctionType
ALU = mybir.AluOpType
F32 = mybir.dt.float32
F32R = mybir.dt.float32r
BF16 = mybir.dt.bfloat16

P = 128
SLAB = 512

B, T, C, H, D = 4, 2048, 1024, 16, 64
N_CORES = 8
N_GROUPS = 2          # head groups (tensor-parallel degree per batch)
HL = H // N_GROUPS    # heads per core
CL = HL * D           # local qkv width


def _build_nc(loop_reps=None):
    NCK = C // P
    MQK = 2 * CL // P
    MQ = MQK // 2
    TT = T // P
    NS = T // SLAB
    YC = CL // P
    W_OUT = min(SLAB, C)
    NOUT = C // W_OUT
    scale = 1.0 / np.sqrt(D)

    nc = bacc.Bacc("TRN2", target_bir_lowering=False, debug=False,
                   num_devices=N_CORES)
    xT = nc.dram_tensor("xT", [C, T], BF16, kind="ExternalInput")
    wqk = nc.dram_tensor("wqk", [C, 2 * CL], BF16, kind="ExternalInput")
    wv = nc.dram_tensor("wv", [C, CL], BF16, kind="ExternalInput")
    wout = nc.dram_tensor("wout", [CL, C], BF16, kind="ExternalInput")
    bqk = nc.dram_tensor("bqk", [P, MQK], F32, kind="ExternalInput")
    bv = nc.dram_tensor("bv", [1, CL], BF16, kind="ExternalInput")
    outp = nc.dram_tensor("outp", [T, C], BF16, kind="ExternalOutput")
    scr = nc.dram_tensor("scr", [HL * NS, SLAB], F32)

    with tile.TileContext(nc) as tc, ExitStack() as ctx:
        pool = lambda name, bufs, **kw: ctx.enter_context(
            tc.tile_pool(name=name, bufs=bufs, **kw))

        const = pool("const", 1)
        kp = pool("kp", 1)
        vp = pool("vp", 1)
        wqkp = pool("wqkp", 1)
        wvp = pool("wvp", 1)
        woutp = pool("woutp", 1)
        xtp = pool("xt", 2)
        qp = pool("qp", 2)
        yTp = pool("yTp", 2)
        expp = pool("expp", 3)
        yfp = pool("yfp", 2)
        nrm_b = pool("nrm_b", 2)
        y8p = pool("y8", 2)
        otp = pool("ot", 2)
        psS = pool("psS", 2, space="PSUM")       # [128, 2, 512] = 2 banks each
        psY = pool("psY", 1, space="PSUM")       # py0 + py1 = 2 banks
        psProj = pool("psProj", 1, space="PSUM")
        psO = pool("psO", 1, space="PSUM")

        k_sb = kp.tile([P, MQ, T], BF16)
        v_sb = vp.tile([P, TT, HL, D + 1], BF16)
        wqk_sb = wqkp.tile([P, NCK, 2 * CL], BF16)
        wv_sb = wvp.tile([P, NCK, CL], BF16)
        wout_sb = woutp.tile([P, YC, C], BF16)
        bqk_sb = const.tile([P, MQK], F32)
        bv_sb = const.tile([1, CL], BF16)
        ident = const.tile([P, P], BF16)
        maskneg = const.tile([P, P], BF16)
        mask01 = const.tile([P, P], F32)
        onescr = const.tile([P, TT * HL], F32)

        nc.sync.dma_start(out=bqk_sb[:, :], in_=bqk[:, :])
        nc.sync.dma_start(out=bv_sb[:, :], in_=bv[:, :])
        for c in range(NCK):
            nc.sync.dma_start(out=wqk_sb[:, c, :], in_=wqk[c * P:(c + 1) * P, :])
            nc.sync.dma_start(out=wv_sb[:, c, :], in_=wv[c * P:(c + 1) * P, :])
        for c in range(YC):
            nc.sync.dma_start(out=wout_sb[:, c, :], in_=wout[c * P:(c + 1) * P, :])
        # mask01[p, f] = 1 if f >= p else 0; maskneg = (mask01 - 1) * 1e30
        # i.e. -1e30 on the strictly-lower (tq < tk) region, 0 elsewhere.
        make_upper_triangular(nc, mask01[:, :], val=1.0, diag=True)
        nc.vector.tensor_scalar(
            maskneg[:, :], mask01[:, :], -1.0, 1e30, op0=ALU.add, op1=ALU.mult)
        make_identity(nc, mask01[:, :])
        nc.vector.tensor_copy(ident[:, :], mask01[:, :])
        # bf16 tiles can't be memset; memset f32 scratch + DVE copy (rounds)
        nc.vector.memset(onescr[:, :], 1.0)
        nc.vector.tensor_copy(
            v_sb[:, :, :, D],
            onescr[:, :].rearrange("p (t h) -> p t h", h=HL))
        ones1 = v_sb[0:1, :, :, D].rearrange("u t h -> u (t h)")

        def body():
            for s in range(NS):
                t0 = s * SLAB
                # ---- A(s): projections for this slab ----
                xt = xtp.tile([P, NCK, SLAB], BF16)
                for c in range(NCK):
                    nc.sync.dma_start(out=xt[:, c, :],
                                      in_=xT[c * P:(c + 1) * P, t0:t0 + SLAB])
                q_sb = qp.tile([P, MQ, SLAB], BF16)
                for m in range(MQK):
                    ps = psProj.tile([P, SLAB], F32, tag="ps")
                    for c in range(NCK):
                        nc.tensor.matmul(
                            ps[:, :],
                            wqk_sb[:, c, m * P:(m + 1) * P],
                            xt[:, c, :],
                            start=(c == 0), stop=(c == NCK - 1))
                    dst = (q_sb[:, m, :] if m < MQ
                           else k_sb[:, m - MQ, t0:t0 + SLAB])
                    sc = scale if m < MQ else 1.0
                    nc.vector.tensor_scalar(
                        dst, ps[:, :], sc, bqk_sb[:, m:m + 1],
                        op0=ALU.mult, op1=ALU.add)
                for sub in range(SLAB // P):
                    tt = s * (SLAB // P) + sub
                    ps = psProj.tile([P, CL], F32, tag="ps")
                    for c in range(NCK):
                        nc.tensor.matmul(
                            ps[:, :],
                            xt[:, c, sub * P:(sub + 1) * P],
                            wv_sb[:, c, :],
                            start=(c == 0), stop=False)
                    nc.tensor.matmul(
                        ps[:, :], ones1[:, :],
                        bv_sb[0:1, :], start=False, stop=True)
                    nc.vector.tensor_copy(
                        v_sb[:, tt, :, 0:D],
                        ps[:, :].rearrange("p (h d) -> p h d", d=D))

                # ---- B: attention; even/odd heads of a pair share the PE
                # array via tile_position row groups (concurrent K=64) and
                # share one 2-bank PSUM S-tile so one Exp covers both ----
                yT_sb = yTp.tile([P, YC, SLAB], BF16)
                for hp in range(HL // 2):
                    nblk = (s + 1) * SLAB // P
                    py0 = psY.tile([D + 1, SLAB], F32, tag="py0")
                    py1 = psY.tile([D + 1, SLAB], F32, tag="py1")
                    pys = (py0, py1)
                    for b in range(nblk):
                        tk0 = b * P
                        off = tk0 - t0
                        vis = max(0, off)
                        ss = psS.tile([P, 2, SLAB], F32)
                        for i in range(2):
                            row0 = i * 64
                            nc.tensor.matmul(
                                ss[:, i, vis:SLAB],
                                k_sb[row0:row0 + 64, hp, tk0:tk0 + P],
                                q_sb[row0:row0 + 64, hp, vis:SLAB],
                                start=True, stop=(off < 0),
                                tile_position=(row0, 0))
                        if off >= 0:
                            for i in range(2):
                                nc.tensor.matmul(
                                    ss[:, i, off:off + P],
                                    ident[:, :], maskneg[:, :],
                                    start=False, stop=True)
                        ep = expp.tile([P, 2, SLAB], BF16)
                        nc.scalar.activation(ep[:, :, vis:SLAB],
                                             ss[:, :, vis:SLAB], AF.Exp)
                        for i in range(2):
                            nc.tensor.matmul(
                                pys[i][0:D + 1, vis:SLAB],
                                v_sb[:, b, 2 * hp + i, 0:D + 1],
                                ep[:, i, vis:SLAB],
                                start=(b == 0), stop=(b == nblk - 1))
                    # evict psY to SBUF right away so the next pair's PV
                    # accumulation doesn't wait on the normalize tail;
                    # reciprocal reads PSUM directly (parallel with the
                    # copy); partition-broadcast of 1/den via DRAM bounce
                    # (stride-0 DRAM source AP).
                    for i in range(2):
                        h = 2 * hp + i
                        row0 = i * 64
                        yf = yfp.tile([D, SLAB], F32, tag="yf")
                        nc.vector.tensor_copy(yf[:, :], pys[i][0:D, :])
                        bi = nrm_b.tile([64, SLAB], F32, tag="binv")
                        nc.vector.reciprocal(bi[0:1, :], pys[i][D:D + 1, :])
                        sidx = h * NS + s
                        nc.sync.dma_start(out=scr[sidx:sidx + 1, :],
                                          in_=bi[0:1, :])
                        src = scr[sidx:sidx + 1, :]
                        bsrc = bass.AP(tensor=src.tensor, offset=src.offset,
                                       ap=[[0, 64], [1, SLAB]])
                        nc.sync.dma_start(out=bi[:, :], in_=bsrc)
                        y8 = y8p.tile([64, SLAB], BF16)
                        nc.vector.tensor_mul(y8[:, :], yf[:, :], bi[:, :])
                        nc.sync.dma_start(
                            out=yT_sb[row0:row0 + 64, hp, :], in_=y8[:, :])

                # ---- C(s): out projection for this slab ----
                for sub in range(SLAB // P):
                    for n in range(NOUT):
                        n0 = n * W_OUT
                        ps = psO.tile([P, W_OUT], F32, tag="ps")
                        for c in range(YC):
                            nc.tensor.matmul(
                                ps[:, :],
                                yT_sb[:, c, sub * P:(sub + 1) * P],
                                wout_sb[:, c, n0:n0 + W_OUT],
                                start=(c == 0), stop=(c == YC - 1))
                        ot = otp.tile([P, W_OUT], BF16)
                        nc.vector.tensor_copy(ot[:, :], ps[:, :])
                        nc.sync.dma_start(
                            out=outp[t0 + sub * P:t0 + (sub + 1) * P,
                                     n0:n0 + W_OUT],
                            in_=ot[:, :])

        if loop_reps is None:
            body()
        else:
            with tc.For_i(0, loop_reps, 1):
                body()

    nc.compile()
    return nc


_NC_CACHE = None


def _get_nc():
    global _NC_CACHE
    if _NC_CACHE is None:
        _NC_CACHE = _build_nc()
    return _NC_CACHE


def make_in_maps(x, W_qkv, b_qkv, W_out):
    import ml_dtypes

    bf = ml_dtypes.bfloat16
    scale = 1.0 / np.sqrt(D)
    MQK = 2 * CL // P
    in_maps = []
    for core in range(N_CORES):
        b, hg = divmod(core, N_GROUPS)
        qs = slice(hg * CL, (hg + 1) * CL)
        ks = slice(C + hg * CL, C + (hg + 1) * CL)
        vs = slice(2 * C + hg * CL, 2 * C + (hg + 1) * CL)
        bqk_cat = np.concatenate([b_qkv[qs] * scale, b_qkv[ks]])
        in_maps.append({
            "xT": np.ascontiguousarray(x[b].T).astype(bf),
            "wqk": np.ascontiguousarray(
                np.concatenate([W_qkv[:, qs], W_qkv[:, ks]], axis=1)).astype(bf),
            "wv": np.ascontiguousarray(W_qkv[:, vs]).astype(bf),
            "wout": np.ascontiguousarray(W_out[hg * CL:(hg + 1) * CL, :]).astype(bf),
            "bqk": np.ascontiguousarray(bqk_cat.reshape(MQK, P).T),
            "bv": np.ascontiguousarray(b_qkv[vs].reshape(1, CL)).astype(bf),
        })
    return in_maps


def kernel(x, W_qkv, b_qkv, W_out, b_out):
    x = np.asarray(x, dtype=np.float32)
    W_qkv = np.asarray(W_qkv, dtype=np.float32)
    b_qkv = np.asarray(b_qkv, dtype=np.float32)
    W_out = np.asarray(W_out, dtype=np.float32)
    b_out = np.asarray(b_out, dtype=np.float32)

    nc = _get_nc()
    in_maps = make_in_maps(x, W_qkv, b_qkv, W_out)
    res = run_bass_kernel_spmd(nc, in_maps, core_ids=list(range(N_CORES)))

    out = np.empty((B, T, C), dtype=np.float32)
    for b in range(B):
        out[b] = (res.results[N_GROUPS * b]["outp"].astype(np.float32)
                  + res.results[N_GROUPS * b + 1]["outp"].astype(np.float32)
                  + b_out)
    return out
